# revision 1
# baseline (speedup 1.0000x reference)
"""Trainium2 Bass kernel for nn_BertEncoder_403726926494.

Reference computation (per batch element):
  - ragged sentence extraction from hidden_states, masked-softmax attention
    pooling per sentence with W_doc            -> doc_pooled [B, D, H]
  - query extraction (rows 1..32), masked-softmax pooling with W_query
    broadcast over D                           -> q_bcast   [B, D, H]

Device strategy (SPMD, one program on 8 cores, 8 batch elements per core):
  - Per core-slot, DMA only the used row-span of hidden_states into SBUF
    (slots are assigned from a global sort of spans so the per-slot span is
    a static program constant shared by all cores).
  - Per-token scores s[t] = x_t . W_doc: DVE/GpSimd tensor_tensor multiply
    against a W-broadcast tile, then a free-dim reduce on ACT (activation
    Copy + accum_out) or DVE (tensor_reduce) -- engine choice per slot to
    balance load.
  - softmax without max-subtraction (scores are O(1)):
      alphaU[t,j] = exp(s[t] + logSel[t,j])   one ACT op per chunk, where
    logSel is a host-built {0, -1e30} mask marking token t in sentence j
    (columns padded to 32 with -1e30).
      num[j,:H] | den[j] = alphaU^T @ [X | 1]  PE matmul with a ones-column
    appended to X; 4 slots share one PSUM tile via tile_position col-groups.
  - out[j] = num[j] / (den[j] + eps)  (eps keeps empty sentences at 0).
  - Query path packs 4 examples x 32 query rows onto 128 partitions; the
    query-length mask and example-block structure fold into one host-built
    log-mask. q_pooled is broadcast over D on the host.
  - b_doc / b_query shift every score in a softmax segment equally, so they
    cancel and are ignored.
"""

import numpy as np

B, L, H = 64, 512, 768
D, S, Q = 16, 64, 32
NCORES = 8
SLOTS = 8
MPAD = 32  # selector columns padded to one PE col-group
NEG_BIAS = -1.0e30
DEN_EPS = 1.0e-30

# Engine assignment knobs (tuned from traces):
#   score TT multiply per slot: "dve" or "gps"
#   score reduce per slot: "act" (per-chunk accum) or "dve" (merged reduce)
TT_ENGINE = ["dve"] * SLOTS
RED_ENGINE = ["act", "act", "act", "act", "act", "act", "dve", "dve"]
Q_RED_ENGINE = "act"

_compiled: dict = {}


def _slot_geometry(slot_spans):
    nts = [(sp + 127) // 128 for sp in slot_spans]
    rems = [sp - 128 * (nt - 1) for sp, nt in zip(slot_spans, nts)]
    coffs = [0]
    for nt in nts:
        coffs.append(coffs[-1] + nt)
    return nts, rems, coffs


def _build(slot_spans):
    """Build + compile the SPMD Bass program for the given per-slot spans."""
    from contextlib import ExitStack

    import concourse.bacc as bacc
    import concourse.tile as tile
    from concourse import mybir

    f32 = mybir.dt.float32
    MULT = mybir.AluOpType.mult
    ADD = mybir.AluOpType.add
    EXP = mybir.ActivationFunctionType.Exp
    COPY = mybir.ActivationFunctionType.Copy

    nts, rems, coffs = _slot_geometry(slot_spans)
    ntsum = coffs[-1]
    foffs = [0]
    for nt in nts:
        foffs.append(foffs[-1] + nt - 1)
    roffs = [0]
    for r in rems:
        roffs.append(roffs[-1] + r)

    nc = bacc.Bacc(
        "TRN2", target_bir_lowering=False, debug=False, num_devices=NCORES
    )
    nfull = sum(nt - 1 for nt in nts)
    nremtot = sum(rems)
    sfull = nc.dram_tensor(
        "sfull", [128, max(nfull, 1), H], f32, kind="ExternalInput"
    ).ap()
    srem = nc.dram_tensor("srem", [nremtot, H], f32, kind="ExternalInput").ap()
    qstage = nc.dram_tensor("qstage", [2, 128, H], f32, kind="ExternalInput").ap()
    wd = nc.dram_tensor("wd", [1, H], f32, kind="ExternalInput").ap()
    wq = nc.dram_tensor("wq", [1, H], f32, kind="ExternalInput").ap()
    selt = nc.dram_tensor(
        "selt", [128, ntsum, MPAD], f32, kind="ExternalInput"
    ).ap()
    qmask = nc.dram_tensor("qmask", [128, 2, MPAD], f32, kind="ExternalInput").ap()
    doc_out = nc.dram_tensor("doc_out", [SLOTS, D, H], f32, kind="ExternalOutput").ap()
    q_out = nc.dram_tensor("q_out", [SLOTS, H], f32, kind="ExternalOutput").ap()

    with tile.TileContext(nc) as tc, ExitStack() as ctx:
        const = ctx.enter_context(tc.tile_pool(name="const", bufs=1))

        wrow_d = const.tile([1, H], f32)
        nc.sync.dma_start(out=wrow_d[:], in_=wd[:])
        wrow_q = const.tile([1, H], f32)
        nc.sync.dma_start(out=wrow_q[:], in_=wq[:])
        selt_t = const.tile([128, ntsum, MPAD], f32)
        nc.sync.dma_start(out=selt_t[:], in_=selt[:])
        qmask_t = const.tile([128, 2, MPAD], f32)
        nc.sync.dma_start(out=qmask_t[:], in_=qmask[:])

        # Broadcast W rows across all 128 partitions (gpsimd custom op).
        wb_d = const.tile([128, H], f32)
        wb_q = const.tile([128, H], f32)
        nc.gpsimd.partition_broadcast(wb_d[:], wrow_d[:])
        nc.gpsimd.partition_broadcast(wb_q[:], wrow_q[:])

        xpool = ctx.enter_context(tc.tile_pool(name="xp", bufs=8))
        apool = ctx.enter_context(tc.tile_pool(name="apl", bufs=4))
        scrp = ctx.enter_context(tc.tile_pool(name="scr", bufs=2))
        outp = ctx.enter_context(tc.tile_pool(name="outp", bufs=2))
        smallp = ctx.enter_context(tc.tile_pool(name="smallp", bufs=4))
        qpoolp = ctx.enter_context(tc.tile_pool(name="qpl", bufs=2))
        nump = ctx.enter_context(tc.tile_pool(name="nump", bufs=2, space="PSUM"))
        qnump = ctx.enter_context(tc.tile_pool(name="qnump", bufs=1, space="PSUM"))

        # ---- scores: xw = x * W_bcast (TT), then free-dim reduce -> scol ----
        def emit_scores(x_ap_full, nt, rem, scol, wb, name, tt_eng, red_eng):
            # x_ap_full: [128, nt, H(+1)] view; uses cols 0:H
            xw = scrp.tile([128, nt, H], f32, tag="scratch", name=f"xw{name}")
            tt = nc.gpsimd if tt_eng == "gps" else nc.vector
            if nt > 1:
                tt.tensor_tensor(
                    out=xw[:, 0 : nt - 1, :],
                    in0=x_ap_full[:, 0 : nt - 1, 0:H],
                    in1=wb[:].rearrange("p (o h) -> p o h", o=1).broadcast_to(
                        [128, nt - 1, H]
                    ),
                    op=MULT,
                )
            tt.tensor_tensor(
                out=xw[0:rem, nt - 1, :],
                in0=x_ap_full[0:rem, nt - 1, 0:H],
                in1=wb[0:rem, :],
                op=MULT,
            )
            if red_eng == "dve":
                if nt > 1:
                    nc.vector.tensor_reduce(
                        out=scol[:, 0 : nt - 1],
                        in_=xw[:, 0 : nt - 1, :],
                        axis=mybir.AxisListType.X,
                        op=ADD,
                    )
                nc.vector.tensor_reduce(
                    out=scol[0:rem, nt - 1 : nt],
                    in_=xw[0:rem, nt - 1, :],
                    axis=mybir.AxisListType.X,
                    op=ADD,
                )
            else:
                s2 = scrp.tile([128, H], f32, tag="scratch2", name=f"s2{name}")
                for c in range(nt):
                    cnt = 128 if c < nt - 1 else rem
                    nc.scalar.activation(
                        s2[0:cnt, :], xw[0:cnt, c, :], COPY,
                        bias=0.0, scale=1.0,
                        accum_out=scol[0:cnt, c : c + 1],
                    )

        # ---- doc slots: per-slot pipeline; two groups of 4 share PSUM tiles
        # via PE col-groups. Slots are emitted alternating between the two
        # groups so independent work overlaps and consecutive slots' matmuls
        # land on different col-groups (concurrent PE streams).
        numgs = {}

        xtiles = {}

        def load_slot(s):
            nt, rem = nts[s], rems[s]
            x = xpool.tile([128, nt, H + 1], f32, tag="x", name=f"x{s}")
            if nt > 1:
                nc.sync.dma_start(
                    out=x[:, 0 : nt - 1, 0:H],
                    in_=sfull[:, foffs[s] : foffs[s] + nt - 1, :],
                )
            nc.sync.dma_start(
                out=x[0:rem, nt - 1, 0:H],
                in_=srem[roffs[s] : roffs[s] + rem, :],
            )
            nc.vector.memset(x[:, :, H : H + 1], 1.0)
            xtiles[s] = x

        def emit_slot(s):
            g, k = divmod(s, 4)
            if g not in numgs:
                numgs[g] = nump.tile([128, 1024], f32, tag="num", name=f"num{g}")
            numg = numgs[g]
            nt, rem, coff = nts[s], rems[s], coffs[s]
            x = xtiles[s]

            scol = smallp.tile([128, nt], f32, tag="scol", name=f"scol{s}")
            emit_scores(
                x[:], nt, rem, scol, wb_d, f"d{s}", TT_ENGINE[s], RED_ENGINE[s]
            )

            at = apool.tile([128, nt, MPAD], f32, tag="at", name=f"at{s}")
            for c in range(nt):
                cnt = 128 if c < nt - 1 else rem
                nc.scalar.activation(
                    at[0:cnt, c, :],
                    selt_t[0:cnt, coff + c, :],
                    EXP,
                    bias=scol[0:cnt, c : c + 1],
                    scale=1.0,
                )
            for c in range(nt):
                cnt = 128 if c < nt - 1 else rem
                first, last = c == 0, c == nt - 1
                nc.tensor.matmul(
                    numg[32 * k : 32 * k + MPAD, 0:512],
                    at[0:cnt, c, :],
                    x[0:cnt, c, 0:512],
                    start=first, stop=last,
                    tile_position=(0, 32 * k),
                    skip_group_check=True,
                )
                nc.tensor.matmul(
                    numg[32 * k : 32 * k + MPAD, 512 : H + 1],
                    at[0:cnt, c, :],
                    x[0:cnt, c, 512 : H + 1],
                    start=first, stop=last,
                    tile_position=(0, 32 * k),
                    skip_group_check=True,
                )

        def finish_group(g):
            numg = numgs[g]
            de = smallp.tile([128, 1], f32, tag="de", name=f"de{g}")
            nc.vector.tensor_scalar(
                out=de[:], in0=numg[:, H : H + 1], scalar1=DEN_EPS,
                scalar2=None, op0=ADD,
            )
            rec = smallp.tile([128, 1], f32, tag="rec", name=f"rec{g}")
            nc.vector.reciprocal(rec[:], de[:])
            do = outp.tile([128, H], f32, tag="do", name=f"do{g}")
            nc.scalar.activation(
                do[:], numg[:, 0:H], COPY, bias=0.0, scale=rec[:, 0:1]
            )
            for k in range(4):
                nc.scalar.dma_start(
                    out=doc_out[4 * g + k, :, :],
                    in_=do[32 * k : 32 * k + D, :],
                )

        # ---- query: two batches of 4 examples x 32 rows -> one PSUM tile ----
        def emit_query(qnumg, b):
            qpack = qpoolp.tile([128, H + 1], f32, tag="qpack", name=f"qpack{b}")
            nc.sync.dma_start(out=qpack[:, 0:H], in_=qstage[b, :, :])
            nc.vector.memset(qpack[:, H : H + 1], 1.0)
            qscol = smallp.tile([128, 1], f32, tag="qscol", name=f"qscol{b}")
            emit_scores(
                qpack[:].rearrange("p (o h) -> p o h", o=1), 1, 128, qscol, wb_q,
                f"q{b}", "dve", Q_RED_ENGINE,
            )
            qat = apool.tile([128, MPAD], f32, tag="qat", name=f"qat{b}")
            nc.scalar.activation(
                qat[:], qmask_t[:, b, :], EXP, bias=qscol[:, 0:1], scale=1.0
            )
            nc.tensor.matmul(
                qnumg[32 * b : 32 * b + MPAD, 0:512],
                qat[:], qpack[:, 0:512],
                start=True, stop=True, tile_position=(0, 32 * b),
            )
            nc.tensor.matmul(
                qnumg[32 * b : 32 * b + MPAD, 512 : H + 1],
                qat[:], qpack[:, 512 : H + 1],
                start=True, stop=True, tile_position=(0, 32 * b),
            )

        qnumg = qnump.tile([64, 1024], f32, tag="qnum", name="qnum")
        for s in range(SLOTS):
            load_slot(s)
        for s in (0, 4, 1, 5):
            emit_slot(s)
        emit_query(qnumg, 0)
        for s in (2, 6, 3, 7):
            emit_slot(s)
        emit_query(qnumg, 1)
        finish_group(0)
        finish_group(1)

        qde = smallp.tile([64, 1], f32, tag="qde", name="qde")
        nc.vector.tensor_scalar(
            out=qde[:], in0=qnumg[:, H : H + 1], scalar1=DEN_EPS,
            scalar2=None, op0=ADD,
        )
        qrec = smallp.tile([64, 1], f32, tag="qrec", name="qrec")
        nc.vector.reciprocal(qrec[:], qde[:])
        qo = outp.tile([64, H], f32, tag="qo", name="qo")
        nc.scalar.activation(
            qo[:], qnumg[:, 0:H], COPY, bias=0.0, scale=qrec[:, 0:1]
        )
        for b in range(2):
            nc.sync.dma_start(
                out=q_out[4 * b : 4 * b + 4, :],
                in_=qo[32 * b : 32 * b + 4, :],
            )

    nc.compile()
    return nc


def _prepare(query_len, seq_lens):
    """Host-side geometry: spans, slot assignment, selector/mask arrays."""
    ql = np.asarray(query_len).astype(np.int64)
    sl = np.asarray(seq_lens).astype(np.int64)
    offs = ql[:, None] + 2 + np.cumsum(sl, axis=1) - sl  # [B, D] sentence starts
    end = ql + 2 + sl.sum(axis=1)
    span = np.maximum(end, 1 + Q)  # query rows 1..32 must be covered
    order = np.argsort(-span, kind="stable")  # rank -> example id
    slot_spans = tuple(int(span[order[8 * s]]) for s in range(SLOTS))
    nts, rems, coffs = _slot_geometry(slot_spans)
    ntsum = coffs[-1]

    selt_all = np.full((NCORES, 128, ntsum, MPAD), NEG_BIAS, np.float32)
    qmask_all = np.full((NCORES, 128, 2, MPAD), NEG_BIAS, np.float32)
    ex_map = np.empty((NCORES, SLOTS), np.int64)
    for c in range(NCORES):
        for s in range(SLOTS):
            e = int(order[8 * s + c])
            ex_map[c, s] = e
            for j in range(D):
                ln = int(sl[e, j])
                if ln == 0:
                    continue
                o = int(offs[e, j])
                t = np.arange(o, o + ln)
                selt_all[c, t % 128, coffs[s] + t // 128, j] = 0.0
            b, sub = divmod(s, 4)
            qmask_all[c, 32 * sub : 32 * sub + int(ql[e]), b, sub] = 0.0
    return slot_spans, ex_map, selt_all, qmask_all


def kernel(hidden_states, W_doc, b_doc, W_query, b_query, query_len, seq_lens):
    hs = np.ascontiguousarray(np.asarray(hidden_states, dtype=np.float32))
    wd = np.ascontiguousarray(np.asarray(W_doc, np.float32).reshape(1, H))
    wq = np.ascontiguousarray(np.asarray(W_query, np.float32).reshape(1, H))

    slot_spans, ex_map, selt_all, qmask_all = _prepare(query_len, seq_lens)

    nc = _compiled.get(slot_spans)
    if nc is None:
        nc = _build(slot_spans)
        _compiled[slot_spans] = nc

    nts, rems, _ = _slot_geometry(slot_spans)
    nfull = sum(nt - 1 for nt in nts)
    nremtot = sum(rems)

    in_maps = []
    for c in range(NCORES):
        sfull = np.empty((128, max(nfull, 1), H), np.float32)
        srem = np.empty((nremtot, H), np.float32)
        qstage = np.empty((2, 128, H), np.float32)
        fo = ro = 0
        for s in range(SLOTS):
            e = int(ex_map[c, s])
            nt, rem = nts[s], rems[s]
            if nt > 1:
                sfull[:, fo : fo + nt - 1, :] = (
                    hs[e, 0 : (nt - 1) * 128, :]
                    .reshape(nt - 1, 128, H)
                    .transpose(1, 0, 2)
                )
                fo += nt - 1
            srem[ro : ro + rem] = hs[e, (nt - 1) * 128 : (nt - 1) * 128 + rem, :]
            ro += rem
            b, sub = divmod(s, 4)
            qstage[b, 32 * sub : 32 * sub + 32, :] = hs[e, 1 : 1 + Q, :]
        in_maps.append(
            {
                "sfull": sfull,
                "srem": srem,
                "qstage": qstage,
                "wd": wd,
                "wq": wq,
                "selt": selt_all[c],
                "qmask": qmask_all[c],
            }
        )

    from concourse.bass_utils import run_bass_kernel_spmd

    res = run_bass_kernel_spmd(nc, in_maps, list(range(NCORES)))

    doc = np.empty((B, D, H), np.float32)
    qp = np.empty((B, H), np.float32)
    for c in range(NCORES):
        r = res.results[c]
        for s in range(SLOTS):
            e = int(ex_map[c, s])
            doc[e] = r["doc_out"][s]
            qp[e] = r["q_out"][s]
    q_bcast = np.broadcast_to(qp[:, None, :], (B, D, H))
    return doc, q_bcast



# revision 5
# speedup vs baseline: 2.2868x; 2.2868x over previous
"""Trainium2 Bass kernel for nn_BertEncoder_403726926494.

Reference computation (per batch element):
  - ragged sentence extraction from hidden_states, masked-softmax attention
    pooling per sentence with W_doc            -> doc_pooled [B, D, H]
  - query extraction (rows 1..32), masked-softmax pooling with W_query
    broadcast over D                           -> q_bcast   [B, D, H]

Device strategy (SPMD, one program on 8 cores, 8 batch elements per core):
  - The host packs, per core, a dense token stream: 8x32 query rows
    (chunks 0..1) followed by each example's contiguous sentence region
    rows [ql+2, ql+2+sum(seq_lens)), concatenated back-to-back, zero-pad
    to T*128 rows.  Stored bf16 as [128, T, 770]: col 768 = 1.0 (ones
    column for the softmax denominators), col 769 = pad.
  - A host-built 0/1 selector sel[token, m] (bf16, [128, T, 128]) maps
    every token to its output row m: cols 0..7 = the 8 queries (masked
    to ql), cols 8.. = every non-empty sentence of the core's examples.
  - Per chunk c (128 tokens):
      scores  s = x . W  (W_query for chunks 0..1, W_doc for the rest)
              one DVE tensor_tensor_reduce (mult + add-accum)
      es      = exp(s)            (ACT, [128,1] per chunk)
      at      = sel * es          (DVE tensor_scalar, per-partition scalar)
      acc    += at^T @ x[:, 0:769]  (PE matmul, K=128, M=128, N=769,
              accumulated over all T chunks in one PSUM region; the ones
              column makes acc[:, 768] the softmax denominator)
  - out[m] = acc[m, 0:768] / (acc[m, 768] + eps); eps keeps unused rows
    at 0.  One [128, 768] bf16 store; host scatters rows to (e, j)/query.
  - b_doc / b_query shift every score in a softmax segment equally, so
    they cancel and are ignored.  exp() without max-subtraction is safe:
    |s| <~ 3 for this data distribution.
"""

import numpy as np
import ml_dtypes

B, L, H = 64, 512, 768
D, S, Q = 16, 64, 32
NCORES = 8
EX_PER_CORE = 8
QCH = 2  # query chunks: 8 examples x 32 rows = 256 = 2*128
HP = H + 2  # 770: ones col at 768, pad col at 769 (4-byte-aligned chunks)
DEN_EPS = 1.0e-30
BF16 = ml_dtypes.bfloat16
SPLIT_MM = True  # True: split each chunk matmul at col 512 (PSUM banks)

_compiled: dict = {}


def _build(T):
    from contextlib import ExitStack

    import concourse.bacc as bacc
    import concourse.tile as tile
    from concourse import mybir

    f32 = mybir.dt.float32
    bf16 = mybir.dt.bfloat16
    MULT = mybir.AluOpType.mult
    ADD = mybir.AluOpType.add
    EXP = mybir.ActivationFunctionType.Exp
    COPY = mybir.ActivationFunctionType.Copy

    nc = bacc.Bacc(
        "TRN2", target_bir_lowering=False, debug=False, num_devices=NCORES
    )

    xs_d = nc.dram_tensor("xs", [128, T, HP], bf16, kind="ExternalInput").ap()
    sel_d = nc.dram_tensor("sel", [128, T, 128], bf16, kind="ExternalInput").ap()
    wb_d = nc.dram_tensor("wb", [128, 2, H], bf16, kind="ExternalInput").ap()
    out_d = nc.dram_tensor("out", [128, H], bf16, kind="ExternalOutput").ap()

    with tile.TileContext(nc) as tc, ExitStack() as ctx:
        const = ctx.enter_context(tc.tile_pool(name="const", bufs=1))
        wb = const.tile([128, 2, H], bf16)
        nc.scalar.dma_start(out=wb[:], in_=wb_d[:])
        sel = const.tile([128, T, 128], bf16)
        nc.scalar.dma_start(out=sel[:], in_=sel_d[:])

        xp = ctx.enter_context(tc.tile_pool(name="xp", bufs=1))
        xs = xp.tile([128, T, HP], bf16)
        for c0 in range(0, T, 2):
            c1 = min(c0 + 2, T)
            nc.sync.dma_start(out=xs[:, c0:c1, :], in_=xs_d[:, c0:c1, :])

        sc = ctx.enter_context(tc.tile_pool(name="sc", bufs=1))
        scol = sc.tile([128, T], f32)
        es = sc.tile([128, T], f32)
        at = sc.tile([128, T, 128], bf16)
        xw = sc.tile([128, H], bf16)
        psum = ctx.enter_context(tc.tile_pool(name="ps", bufs=1, space="PSUM"))
        acc = psum.tile([128, 1024], f32)

        for c in range(T):
            w = wb[:, 1 if c < QCH else 0, :]
            nc.vector.scalar_tensor_tensor(
                out=xw[:],
                in0=xs[:, c, 0:H],
                scalar=1.0,
                in1=w,
                op0=MULT,
                op1=MULT,
                accum_out=scol[:, c : c + 1],
            )
            nc.scalar.activation(es[:, c : c + 1], scol[:, c : c + 1], EXP)
            nc.vector.tensor_scalar(
                out=at[:, c, :],
                in0=sel[:, c, :],
                scalar1=es[:, c : c + 1],
                scalar2=None,
                op0=MULT,
            )
            first, last = c == 0, c == T - 1
            if SPLIT_MM:
                nc.tensor.matmul(
                    acc[:, 0:512], at[:, c, :], xs[:, c, 0:512],
                    start=first, stop=last,
                )
                nc.tensor.matmul(
                    acc[:, 512 : H + 1], at[:, c, :], xs[:, c, 512 : H + 1],
                    start=first, stop=last,
                )
            else:
                nc.tensor.matmul(
                    acc[:, 0 : H + 1], at[:, c, :], xs[:, c, 0 : H + 1],
                    start=first, stop=last,
                )

        de = sc.tile([128, 1], f32)
        nc.vector.tensor_scalar(
            out=de[:], in0=acc[:, H : H + 1], scalar1=DEN_EPS,
            scalar2=None, op0=ADD,
        )
        rec = sc.tile([128, 1], f32)
        nc.vector.reciprocal(rec[:], de[:])
        do = sc.tile([128, H], bf16)
        nc.scalar.activation(do[:], acc[:, 0:H], COPY, bias=0.0, scale=rec[:, 0:1])
        nc.scalar.dma_start(out=out_d[:], in_=do[:])

    nc.compile()
    return nc


def _prepare(query_len, seq_lens):
    """Assign examples to cores (balanced stream length) and compute T."""
    ql = np.asarray(query_len).astype(np.int64)
    sl = np.asarray(seq_lens).astype(np.int64)
    S = sl.sum(axis=1)
    dl = (sl > 0).sum(axis=1)
    order = np.argsort(-S, kind="stable")
    loads = np.zeros(NCORES, np.int64)
    counts = np.zeros(NCORES, np.int64)
    assign = [[] for _ in range(NCORES)]
    for e in order:
        cand = [c for c in range(NCORES) if counts[c] < EX_PER_CORE]
        c = min(cand, key=lambda k: loads[k])
        assign[c].append(int(e))
        loads[c] += int(S[e])
        counts[c] += 1
    T = QCH + int((int(loads.max()) + 127) // 128)
    for c in range(NCORES):
        m_used = EX_PER_CORE + int(dl[assign[c]].sum())
        assert m_used <= 128, f"core {c} needs {m_used} output rows"
    return assign, T, ql, sl, S, dl


def _pack_core(hs, examples, T, ql, sl, S, dl):
    """Build one core's packed stream, selector and output-row maps."""
    rows = T * 128
    xsh = np.zeros((rows, HP), np.float32)
    xsh[:, H] = 1.0
    sel = np.zeros((rows, 128), np.float32)
    sent_rows = {}  # (e, j) -> output row m
    q_rows = {}  # e -> output row m
    mcol = EX_PER_CORE
    pos = QCH * 128
    for i, e in enumerate(examples):
        r0 = 32 * i
        xsh[r0 : r0 + 32, 0:H] = hs[e, 1 : 1 + Q, :]
        sel[r0 : r0 + int(ql[e]), i] = 1.0
        q_rows[e] = i
        ns = int(S[e])
        base = int(ql[e]) + 2
        xsh[pos : pos + ns, 0:H] = hs[e, base : base + ns, :]
        off = 0
        for j in range(int(dl[e])):
            ln = int(sl[e, j])
            sel[pos + off : pos + off + ln, mcol] = 1.0
            sent_rows[(e, j)] = mcol
            off += ln
            mcol += 1
        pos += ns
    xs = np.ascontiguousarray(
        xsh.reshape(T, 128, HP).transpose(1, 0, 2)
    ).astype(BF16)
    selr = np.ascontiguousarray(
        sel.reshape(T, 128, 128).transpose(1, 0, 2)
    ).astype(BF16)
    return xs, selr, q_rows, sent_rows


def kernel(hidden_states, W_doc, b_doc, W_query, b_query, query_len, seq_lens):
    hs = np.ascontiguousarray(np.asarray(hidden_states, dtype=np.float32))
    wd = np.asarray(W_doc, np.float32).reshape(H)
    wq = np.asarray(W_query, np.float32).reshape(H)

    assign, T, ql, sl, S, dl = _prepare(query_len, seq_lens)

    nc = _compiled.get(T)
    if nc is None:
        nc = _build(T)
        _compiled[T] = nc

    wb = np.empty((128, 2, H), np.float32)
    wb[:, 0, :] = wd[None, :]
    wb[:, 1, :] = wq[None, :]
    wb = wb.astype(BF16)

    in_maps = []
    maps = []
    for c in range(NCORES):
        xs, selr, q_rows, sent_rows = _pack_core(
            hs, assign[c], T, ql, sl, S, dl
        )
        in_maps.append({"xs": xs, "sel": selr, "wb": wb})
        maps.append((q_rows, sent_rows))

    from concourse.bass_utils import run_bass_kernel_spmd

    res = run_bass_kernel_spmd(nc, in_maps, list(range(NCORES)))

    doc = np.zeros((B, D, H), np.float32)
    qp = np.empty((B, H), np.float32)
    for c in range(NCORES):
        r = np.asarray(res.results[c]["out"], dtype=np.float32)
        q_rows, sent_rows = maps[c]
        for e, m in q_rows.items():
            qp[e] = r[m]
        for (e, j), m in sent_rows.items():
            doc[e, j] = r[m]
    q_bcast = np.broadcast_to(qp[:, None, :], (B, D, H))
    return doc, q_bcast
